# revision 19
# baseline (speedup 1.0000x reference)
"""Decoder kernel for trn2 — full pipeline.

Sharding: data-parallel over batch N=64 -> 8 sequences per core.
Per core:
  B. attention (energy/exp-mask/context) per sequence  [f32r matmuls]
  C. z1x = [ce|ctx/s] @ W_ih1p.T (+b1) batched, n-major rows -> DRAM (bf16)
  D. LSTM chain1+chain2 staggered; 4 col groups (bf16 matmuls);
     gates feature-sliced per group; partition-shifted ACT repack
  E. vocab projection (bf16), l-major rows, streamed w_outT
"""
import sys
sys.path.insert(0, '/opt/trn_rl_repo')
import numpy as np
import ml_dtypes
from concourse import bass, bacc, mybir
from concourse.tile import TileContext
from concourse import masks

F32, F32R, BF16 = mybir.dt.float32, mybir.dt.float32r, mybir.dt.bfloat16
AF = mybir.ActivationFunctionType
ALU = mybir.AluOpType

N_CORES = 8
T, K, V, H, L, VOCAB = 512, 512, 512, 512, 256, 10000
NL = 8
R = NL * L       # 2048 rows per core
MASK_NEG = -30.0
VB = 500         # vocab slice (20 slices)


PERM = np.concatenate([
    np.concatenate([gate * 512 + np.arange(128 * j, 128 * (j + 1))
                    for gate in (0, 1, 3, 2)])       # i, f, o, g
    for j in range(4)])


def _bf(x):
    return np.ascontiguousarray(x).astype(ml_dtypes.bfloat16)


def host_prep(inputs):
    key = np.asarray(inputs["key"], np.float32)
    values = np.asarray(inputs["values"], np.float32)
    text = np.asarray(inputs["text"])
    text_lens = np.asarray(inputs["text_lens"])
    emb = np.asarray(inputs["emb"], np.float32)
    w_ih1 = np.asarray(inputs["w_ih1"], np.float32)
    w_hh1 = np.asarray(inputs["w_hh1"], np.float32)
    w_ih2 = np.asarray(inputs["w_ih2"], np.float32)
    w_hh2 = np.asarray(inputs["w_hh2"], np.float32)
    b1 = (np.asarray(inputs["b_ih1"], np.float32)
          + np.asarray(inputs["b_hh1"], np.float32))
    b2 = (np.asarray(inputs["b_ih2"], np.float32)
          + np.asarray(inputs["b_hh2"], np.float32))
    w_out = np.asarray(inputs["w_out"], np.float32)

    w_ih1p, w_hh1p, b1p = w_ih1[PERM], w_hh1[PERM], b1[PERM]
    w_ih2p, w_hh2p, b2p = w_ih2[PERM], w_hh2[PERM], b2[PERM]

    mask = (np.arange(T)[None, :] < text_lens[:, None])
    maskb = np.where(mask, 0.0, MASK_NEG).astype(np.float32)

    ce_all = emb[text[:, :L]]

    shared = {
        "w1ceT": np.ascontiguousarray(w_ih1p[:, :512].T),
        "w1ctxT": np.ascontiguousarray(w_ih1p[:, 512:].T),
        "b1": b1p.reshape(1, 2048),
        "b2": b2p.reshape(1, 2048),
        "whh1T": _bf(w_hh1p.T),          # (512, 2048) bf16
        "wih2T": _bf(w_ih2p.T),
        "whh2T": _bf(w_hh2p.T),
        "b2bf": _bf(b2p.reshape(1, 2048)),
        "w_outT": _bf(w_out.T),          # (1024, 10000) bf16
    }
    in_maps = []
    for c in range(N_CORES):
        sl = slice(8 * c, 8 * c + 8)
        ceT = np.ascontiguousarray(ce_all[sl].reshape(R, H).T)
        keyT = np.ascontiguousarray(key[:, sl, :].transpose(1, 2, 0))
        vals = np.ascontiguousarray(values[:, sl, :])
        valT = _bf(values[:L, sl, :].reshape(R, V).T)
        m = {"ceT": ceT, "keyT": keyT, "vals": vals, "valT": valT,
             "maskb": np.ascontiguousarray(maskb[sl]), **shared}
        in_maps.append(m)
    return in_maps


def build(debug_outputs=(), upto="E", with_b1=True, with_b2=False, reps=1):
    nc = bacc.Bacc("TRN2", target_bir_lowering=False, debug=False,
                   num_devices=N_CORES)
    d = {}
    def din(name, shape, dt=F32):
        d[name] = nc.dram_tensor(name, list(shape), dt, kind="ExternalInput")
    din("ceT", (H, R)); din("keyT", (NL, K, T)); din("vals", (T, NL, V))
    din("maskb", (NL, T))  # L-indep
    din("w1ceT", (512, 2048)); din("w1ctxT", (512, 2048))
    din("b1", (1, 2048)); din("b2", (1, 2048))
    din("whh1T", (512, 2048), BF16); din("wih2T", (512, 2048), BF16)
    din("whh2T", (512, 2048), BF16); din("b2bf", (1, 2048), BF16)
    din("valT", (V, R), BF16); din("w_outT", (1024, VOCAB), BF16)

    out = nc.dram_tensor("out", [NL, L, VOCAB], F32, kind="ExternalOutput")
    dbg = {}
    shapes = {"att": (NL, T, L), "ctxT": (V, R), "recip": (NL, L),
              "z1x": (L, NL, 2048), "hh": (L, 2 * NL, H)}
    for name in debug_outputs:
        dbg[name] = nc.dram_tensor("dbg_" + name, list(shapes[name]), F32,
                                   kind="ExternalOutput")

    with TileContext(nc) as tc:
        if reps > 1:
            with tc.For_i(0, reps):
                build_body(nc, tc, d, out, dbg, upto, with_b1, with_b2)
        else:
            build_body(nc, tc, d, out, dbg, upto, with_b1, with_b2)
    return nc


def build_body(nc, tc, d, out, dbg, upto, with_b1, with_b2):
    from contextlib import ExitStack
    ctx = ExitStack()
    pool = ctx.enter_context(tc.tile_pool(name="main", bufs=1))
    dramp = ctx.enter_context(tc.tile_pool(name="drp", bufs=1, space="DRAM"))

    # ---- constants ----
    ones_f = pool.tile([128, 128], F32, tag="ones_f")
    nc.gpsimd.memset(ones_f[:], 1.0)
    ones_row = pool.tile([1, 128], F32R, tag="ones_row")
    nc.vector.tensor_copy(ones_row[:], ones_f[0:1, :])
    ones_col = pool.tile([128, 1], F32R, tag="ones_col")
    nc.vector.tensor_copy(ones_col[:], ones_f[:, 0:1])
    ident8f = pool.tile([8, 8], F32, tag="ident8f")
    masks.make_identity(nc, ident8f[:])
    ident8b = pool.tile([8, 8], BF16, tag="ident8b")
    masks.make_identity(nc, ident8b[:])
    ident16f = pool.tile([16, 16], F32, tag="ident16f")
    masks.make_identity(nc, ident16f[:])
    onesb_row = pool.tile([1, 8], BF16, tag="onesb_row")
    nc.gpsimd.memset(onesb_row[:], 1.0)

    z1x_dram = dramp.tile([L, NL, 2048], BF16, tag="z1x_dram")

    # ================= Phase B/C scope =================
    from contextlib import ExitStack as _ES
    with tc.tile_pool(name="bc", bufs=1) as bcp, \
         tc.tile_pool(name="bcps", bufs=4, space="PSUM") as psum, \
         _ES() as bstack:
        attp = bstack.enter_context(tc.tile_pool(name="attp", bufs=2))
        ceT_sb = [bcp.tile([128, R], F32R, tag=f"ceT{kc}", name=f"ceT{kc}")
                  for kc in range(4)]
        for kc in range(4):
            nc.sync.dma_start(ceT_sb[kc][:],
                              d["ceT"][128*kc:128*(kc+1), :].bitcast(F32R))
        ctxT_sb = [bcp.tile([128, R], F32R, tag=f"ctxT{vc}", name=f"ctxT{vc}")
                   for vc in range(4)]
        recipT_sb = bcp.tile([128, 16], F32, tag="recipT")
        sums_sb = bcp.tile([1, NL * L], F32, tag="sums")

        # ---- Phase B: attention ----
        for n in range(NL):
            keyT_n = attp.tile([128, 4 * T], F32R, tag="keyT_n")
            for kc in range(4):
                nc.sync.dma_start(keyT_n[:, T*kc:T*(kc+1)],
                                  d["keyT"][n, 128*kc:128*(kc+1), :].bitcast(F32R))
            maskb_n = attp.tile([128, 4], F32, tag="maskb_n")
            nc.sync.dma_start(maskb_n[:],
                              d["maskb"][n, :].rearrange("(a b) -> b a", b=128))
            att_n = attp.tile([128, 4 * L], F32R, tag="att_n")
            for tch in range(4):
                ep = psum.tile([128, L], F32, tag="mm")
                for kc in range(4):
                    nc.tensor.matmul(ep[:],
                                     keyT_n[:, T*kc+128*tch:T*kc+128*tch+128],
                                     ceT_sb[kc][:, L*n:L*(n+1)],
                                     start=(kc == 0), stop=(kc == 3))
                nc.scalar.activation(att_n[:, L*tch:L*(tch+1)], ep[:], AF.Exp,
                                     bias=maskb_n[:, tch:tch+1])
            sp = psum.tile([1, L], F32, tag="sp", bufs=1)
            for tch in range(4):
                nc.tensor.matmul(sp[:], ones_col[:, :1],
                                 att_n[:, L*tch:L*(tch+1)],
                                 start=(tch == 0), stop=(tch == 3))
            nc.scalar.activation(sums_sb[0:1, L*n:L*(n+1)], sp[:], AF.Copy)
            vals_n = attp.tile([128, 4 * V], F32R, tag="vals_n")
            for tch in range(4):
                nc.sync.dma_start(vals_n[:, V*tch:V*(tch+1)],
                                  d["vals"][128*tch:128*(tch+1), n, :].bitcast(F32R))
            for vc in range(4):
                cp = psum.tile([128, L], F32, tag="mm")
                for tch in range(4):
                    nc.tensor.matmul(cp[:],
                                     vals_n[:, V*tch+128*vc:V*tch+128*vc+128],
                                     att_n[:, L*tch:L*(tch+1)],
                                     start=(tch == 0), stop=(tch == 3))
                nc.scalar.activation(ctxT_sb[vc][:, L*n:L*(n+1)], cp[:], AF.Copy)
            if "att" in dbg:
                af = attp.tile([128, 4 * L], F32, tag="dbgf", bufs=1)
                nc.vector.tensor_copy(af[:], att_n[:].bitcast(F32))
                for tch in range(4):
                    nc.sync.dma_start(dbg["att"][n, 128*tch:128*(tch+1), :],
                                      af[:, L*tch:L*(tch+1)])

        recip_nb = bcp.tile([1, NL * L], F32, tag="recip_nb")
        nc.vector.reciprocal(recip_nb[:], sums_sb[:])
        rcols = min(L, 128)
        nchk = max(L // 128, 1)
        for n in range(NL):
            for hh in range(nchk):
                rp = psum.tile([128, 1], F32, tag="rp", bufs=2)
                nc.tensor.matmul(rp[:rcols, :],
                                 recip_nb[0:1, L*n+rcols*hh:L*n+rcols*(hh+1)],
                                 ident8f[0:1, 0:1], is_transpose=True)
                nc.scalar.activation(recipT_sb[:rcols, 8*hh+n:8*hh+n+1],
                                     rp[:rcols, :], AF.Copy)
        if "recip" in dbg:
            nc.sync.dma_start(dbg["recip"][:, :], recip_nb[:])
        if "ctxT" in dbg:
            for vc in range(4):
                cf = attp.tile([128, R], F32, tag="dbgf", bufs=1)
                nc.vector.tensor_copy(cf[:], ctxT_sb[vc][:].bitcast(F32))
                nc.sync.dma_start(dbg["ctxT"][128*vc:128*(vc+1), :], cf[:])
        if upto == "B":
            ctx.close(); return

        # ---- Phase C: z1x (n-major rows; -> z1x_dram step-major bf16) ----
        bstack.close()   # free attention pools before the z1x weights
        zstack = _ES()
        zwp = zstack.enter_context(tc.tile_pool(name="zwp", bufs=1))
        zxp = zstack.enter_context(tc.tile_pool(name="zxp", bufs=4))
        w1ce_sb = [zwp.tile([128, 2048], F32R, tag=f"w1ce{kc}", name=f"w1ce{kc}")
                   for kc in range(4)]
        w1ctx_sb = [zwp.tile([128, 2048], F32R, tag=f"w1ctx{kc}", name=f"w1ctx{kc}")
                    for kc in range(4)]
        b1_sb = zwp.tile([1, 2048], F32R, tag="b1_sb")
        nc.sync.dma_start(b1_sb[:], d["b1"][:].bitcast(F32R))
        for kc in range(4):
            nc.sync.dma_start(w1ce_sb[kc][:],
                              d["w1ceT"][128*kc:128*(kc+1), :].bitcast(F32R))
            nc.sync.dma_start(w1ctx_sb[kc][:],
                              d["w1ctxT"][128*kc:128*(kc+1), :].bitcast(F32R))
        lchunks = max(L // 128, 1)
        crows = min(L, 128)  # rows per (n, lchunk) piece
        for rc in range(R // crows):
            n_of = rc // lchunks
            lh = rc % lchunks
            rs = slice(crows * rc, crows * (rc + 1))
            recip_col = recipT_sb[:crows, 8*lh + n_of: 8*lh + n_of + 1]
            for j in range(4):
                fs = slice(512 * j, 512 * (j + 1))
                pce = psum.tile([128, 512], F32, tag="mm")
                first = True
                if with_b1:
                    nc.tensor.matmul(pce[:crows, :], ones_row[:1, :crows], b1_sb[:1, fs],
                                     start=True, stop=False)
                    first = False
                for kc in range(4):
                    nc.tensor.matmul(pce[:crows, :], ceT_sb[kc][:, rs],
                                     w1ce_sb[kc][:, fs],
                                     start=(first and kc == 0), stop=(kc == 3))
                pctx = psum.tile([128, 512], F32, tag="mm")
                for kc in range(4):
                    nc.tensor.matmul(pctx[:crows, :], ctxT_sb[kc][:, rs],
                                     w1ctx_sb[kc][:, fs],
                                     start=(kc == 0), stop=(kc == 3))
                zce_sb = zxp.tile([128, 512], F32, tag="zce_sb")
                nc.scalar.activation(zce_sb[:crows, :], pce[:crows, :], AF.Copy)
                zx_bf = zxp.tile([128, 512], BF16, tag="zx_bf")
                nc.vector.scalar_tensor_tensor(out=zx_bf[:crows, :], in0=pctx[:crows, :],
                                               scalar=recip_col, in1=zce_sb[:crows, :],
                                               op0=ALU.mult, op1=ALU.add)
                # rows (n_of, l=crows*lh + p) -> z1x_dram[l, n, f]
                nc.sync.dma_start(z1x_dram[crows*lh:crows*(lh+1), n_of, fs], zx_bf[:crows, :])
        if "z1x" in dbg:
            for t in range(0, L, 8):
                zrb = zxp.tile([8 * NL, 2048], BF16, tag="zrb", bufs=1)
                nc.sync.dma_start(zrb[:], z1x_dram[t:t+8, :, :])
                zrf = zxp.tile([8 * NL, 2048], F32, tag="zrf", bufs=1)
                nc.vector.tensor_copy(zrf[:], zrb[:])
                nc.sync.dma_start(dbg["z1x"][t:t+8, :, :], zrf[:])
        zstack.close()
    if upto == "C":
        ctx.close(); return

    # ================= Phase D: staggered LSTM chains =================
    # Split PSUM groups per chain; chain2 lags chain1 by 2 steps so the
    # full-partition ([128, x]) gate math hides under the PE weight streams.
    # Window j (partitions 32j..32j+8) holds rows (seqs) for gate feature
    # block j = [i f o g] x 128 after the host-side PERM.
    h2T_all = pool.tile([128, 4 * R], BF16, tag="h2T_all")   # [p, (kc, r)] r=8t+n
    h2T_v = h2T_all[:].rearrange("p (k r) -> p k r", k=4)

    with tc.tile_pool(name="dph", bufs=1) as dph, \
         tc.tile_pool(name="dst", bufs=3) as dst, \
         tc.tile_pool(name="dps", bufs=2, space="PSUM") as dps, \
         tc.tile_pool(name="hps", bufs=2, space="PSUM") as hps, \
         tc.tile_pool(name="wvp", bufs=16) as wvp, \
         tc.tile_pool(name="osp", bufs=6) as osp, \
         tc.tile_pool(name="eps", bufs=2, space="PSUM") as eps:
        whh1_sb = [dph.tile([128, 2048], BF16, tag=f"whh1_{k}", name=f"whh1_{k}")
                   for k in range(4)]
        wih2_sb = [dph.tile([128, 2048], BF16, tag=f"wih2_{k}", name=f"wih2_{k}")
                   for k in range(4)]
        whh2_sb = [dph.tile([128, 2048], BF16, tag=f"whh2_{k}", name=f"whh2_{k}")
                   for k in range(4)]
        for k in range(4):
            nc.sync.dma_start(whh1_sb[k][:], d["whh1T"][128*k:128*(k+1), :])
            nc.sync.dma_start(wih2_sb[k][:], d["wih2T"][128*k:128*(k+1), :])
            nc.sync.dma_start(whh2_sb[k][:], d["whh2T"][128*k:128*(k+1), :])
        b2_sb = dph.tile([1, 2048], BF16, tag="b2_sb")
        nc.sync.dma_start(b2_sb[:], d["b2bf"][:])

        valT_sb = dph.tile([128, 4 * R], BF16, tag="valT_sb")
        valT_v = valT_sb[:].rearrange("p (k r) -> p k r", k=4)
        for k in range(4):
            nc.sync.dma_start(valT_v[:, k, :], d["valT"][128*k:128*(k+1), :])

        def emit_vocab(rc_lo, rc_hi, tagp):
            for vb in range(VOCAB // VB):
                wts = [wvp.tile([128, VB], BF16, tag="wv",
                                name=f"wv{tagp}_{vb}_{k}") for k in range(8)]
                for k in range(8):
                    nc.sync.dma_start(wts[k][:],
                                      d["w_outT"][128*k:128*(k+1),
                                                  VB*vb:VB*(vb+1)])
                for rc in range(rc_lo, rc_hi):
                    pv = eps.tile([128, VB], F32, tag="pv")
                    for k in range(4):
                        nc.tensor.matmul(pv[:], h2T_v[:, k, 128*rc:128*(rc+1)],
                                         wts[k][:], start=(k == 0), stop=False)
                    for k in range(4):
                        nc.tensor.matmul(pv[:], valT_v[:, k, 128*rc:128*(rc+1)],
                                         wts[4+k][:], start=False,
                                         stop=(k == 3))
                    osb = osp.tile([128, VB], F32, tag="osb")
                    if rc % 2 == 0:
                        nc.scalar.activation(osb[:], pv[:], AF.Copy)
                    else:
                        nc.vector.tensor_copy(osb[:], pv[:])
                    nc.sync.dma_start(
                        out[0:NL, 16*rc:16*(rc+1), VB*vb:VB*(vb+1)]
                        .transpose([1, 0, 2]), osb[:])

        # stationaries: st1 = h1T (shared by whh1 and wih2 streams, 3 slots:
        # chain1 needs h1[s-1], chain2 needs h1[s-2]); st2 = h2T (2 slots).
        # 32-wide per k-chunk, cols 8:32 zero-padded so each MM writes its
        # full 32-row PSUM window (keeps every PSUM byte defined).
        ident128f = dph.tile([128, 128], F32, tag="ident128f")
        masks.make_identity(nc, ident128f[:])
        identpad = dph.tile([8, 32], BF16, tag="identpad")
        nc.gpsimd.memset(identpad[:], 0.0)
        masks.make_identity(nc, identpad[0:8, 0:8], nomemset=True)
        st1 = [dph.tile([128, 128], BF16, tag=f"st1_{i}", name=f"st1_{i}")
               for i in range(3)]
        st2 = [dph.tile([128, 128], BF16, tag=f"st2_{i}", name=f"st2_{i}")
               for i in range(2)]
        for t in st1 + st2:
            nc.gpsimd.memset(t[:], 0.0)
        st1_v = [t[:].rearrange("p (k c) -> p k c", k=4) for t in st1]
        st2_v = [t[:].rearrange("p (k c) -> p k c", k=4) for t in st2]
        cc1 = dph.tile([128, 128], F32, tag="cc1_init")
        nc.vector.memset(cc1[:], 0.0)
        cc2 = dph.tile([128, 128], F32, tag="cc2_init")
        nc.vector.memset(cc2[:], 0.0)

        def gate_math(zp, cc_prev, tg):
            # zp valid rows: 32j..32j+8 per window; other rows compute
            # garbage that is never read (slices below skip them).
            # Column halves are separate PSUM groups: sigmoid(i,f) and
            # f*c start while the o,g half is still streaming.
            ggif = dst.tile([128, 256], F32, tag="gf" + tg, bufs=2)
            nc.scalar.activation(ggif[:], zp[:, 0:256], AF.Sigmoid)
            t1 = dst.tile([128, 128], F32, tag="t1" + tg, bufs=2)
            nc.vector.tensor_tensor(out=t1[:], in0=ggif[:, 128:256],
                                    in1=cc_prev[:], op=ALU.mult)
            gt = dst.tile([128, 128], F32, tag="gt" + tg, bufs=2)
            nc.scalar.activation(gt[:], zp[:, 384:512], AF.Tanh)
            ggo = dst.tile([128, 128], F32, tag="go" + tg, bufs=2)
            nc.scalar.activation(ggo[:], zp[:, 256:384], AF.Sigmoid)
            t2 = dst.tile([128, 128], F32, tag="t2" + tg, bufs=2)
            nc.vector.tensor_tensor(out=t2[:], in0=ggif[:, 0:128],
                                    in1=gt[:], op=ALU.mult)
            cc_new = dst.tile([128, 128], F32, tag="cc" + tg, bufs=2)
            nc.vector.tensor_tensor(out=cc_new[:], in0=t1[:], in1=t2[:],
                                    op=ALU.add)
            tcg = dst.tile([128, 128], F32, tag="tc" + tg, bufs=2)
            nc.scalar.activation(tcg[:], cc_new[:], AF.Tanh)
            hh = dst.tile([128, 128], F32, tag="hh" + tg, bufs=2)
            nc.vector.tensor_tensor(out=hh[:], in0=ggo[:],
                                    in1=tcg[:], op=ALU.mult)
            return hh, cc_new

        hh2_pend = None
        for s in range(L + 3):
            c1_on = s < L             # chain1 computes h1[s]
            c2_on = 2 <= s < L + 2    # chain2 computes h2[s-2]
            # ---- chain1 PE group: z1[s] = z1x[s] + h1[s-1] @ whh1.T ----
            if c1_on:
                z1t = dst.tile([NL, 2048], BF16, tag="z1t", bufs=4)
                nc.sync.dma_start(z1t[:], z1x_dram[s, :, :])
                zp1 = dps.tile([128, 512], F32, tag="zp1", bufs=2)
                for h in range(2):
                    cs = slice(256 * h, 256 * (h + 1))
                    for j in range(4):
                        nc.tensor.matmul(zp1[32*j:32*j+32, cs], identpad[:],
                                         z1t[:, 512*j+256*h:512*j+256*(h+1)],
                                         start=True, stop=False,
                                         skip_group_check=True,
                                         tile_position=(0, 32*j))
                    for k in range(4):
                        for j in range(4):
                            nc.tensor.matmul(
                                zp1[32*j:32*j+32, cs],
                                st1_v[(s - 1) % 3][:, k, :],
                                whh1_sb[k][:, 512*j+256*h:512*j+256*(h+1)],
                                start=False, stop=(k == 3),
                                skip_group_check=True,
                                tile_position=(0, 32*j))
            # ---- deferred chain2 transpose + copies from step s-1 ----
            if hh2_pend is not None:
                hh2p, t2p = hh2_pend
                hp2 = hps.tile([128, 128], F32, tag="hp", bufs=2)
                nc.tensor.transpose(hp2[:], hh2p[:], ident128f[:])
                hp2_v = hp2[:].rearrange("p (k c) -> p k c", k=4)
                nc.scalar.activation(st2_v[(s - 1) % 2][:, :, 0:8],
                                     hp2_v[:, :, 0:8], AF.Copy)
                nc.vector.tensor_copy(h2T_v[:, :, 8*t2p:8*t2p+8],
                                      hp2_v[:, :, 0:8])
                hh2_pend = None
            if s == (L // 2) + 2 and L >= 128:
                emit_vocab(0, R // 256, "a")   # rows for steps < L/2
            # ---- chain1 gate math (hidden under chain2 PE stream) ----
            if c1_on:
                hh1, cc1 = gate_math(zp1, cc1, "1")
                if "hh" in dbg:
                    for j in range(4):
                        nc.sync.dma_start(
                            dbg["hh"][s, 0:8, 128*j:128*(j+1)],
                            hh1[32*j:32*j+8, :])
            # ---- chain2 PE group: z2[s-2] = h1[s-2]@wih2.T + h2[s-3]@whh2.T
            if c2_on:
                zp2 = dps.tile([128, 512], F32, tag="zp2", bufs=2)
                for h in range(2):
                    cs = slice(256 * h, 256 * (h + 1))
                    for k in range(4):
                        for j in range(4):
                            nc.tensor.matmul(
                                zp2[32*j:32*j+32, cs],
                                st1_v[(s - 2) % 3][:, k, :],
                                wih2_sb[k][:, 512*j+256*h:512*j+256*(h+1)],
                                start=(k == 0), stop=False,
                                skip_group_check=True,
                                tile_position=(0, 32*j))
                    if with_b2:
                        for j in range(4):
                            nc.tensor.matmul(
                                zp2[32*j:32*j+8, cs],
                                onesb_row[:1, :8],
                                b2_sb[:1, 512*j+256*h:512*j+256*(h+1)],
                                start=False, stop=False,
                                skip_group_check=True,
                                tile_position=(0, 32*j))
                    for k in range(4):
                        for j in range(4):
                            nc.tensor.matmul(
                                zp2[32*j:32*j+32, cs],
                                st2_v[(s - 1) % 2][:, k, :],
                                whh2_sb[k][:, 512*j+256*h:512*j+256*(h+1)],
                                start=False, stop=(k == 3),
                                skip_group_check=True,
                                tile_position=(0, 32*j))
            # ---- chain1 transpose + stationary copy ----
            if c1_on:
                hp1 = hps.tile([128, 128], F32, tag="hp", bufs=2)
                nc.tensor.transpose(hp1[:], hh1[:], ident128f[:])
                hp1_v = hp1[:].rearrange("p (k c) -> p k c", k=4)
                nc.scalar.activation(st1_v[s % 3][:, :, 0:8],
                                     hp1_v[:, :, 0:8], AF.Copy)
            # ---- chain2 gate math (hidden under next step's c1 stream) ----
            if c2_on:
                hh2, cc2 = gate_math(zp2, cc2, "2")
                hh2_pend = (hh2, s - 2)
                if "hh" in dbg:
                    for j in range(4):
                        nc.sync.dma_start(
                            dbg["hh"][s-2, 8:16, 128*j:128*(j+1)],
                            hh2[32*j:32*j+8, :])
        emit_vocab(R // 256 if L >= 128 else 0, R // 128, "b")
    if upto == "D":
        ctx.close(); return

    ctx.close()


_CACHE = {}


def _get_runner(with_b1, with_b2, reps=1):
    key = (with_b1, with_b2, reps)
    if key in _CACHE:
        return _CACHE[key]
    import jax
    from jax.sharding import Mesh, PartitionSpec
    from jax.experimental.shard_map import shard_map
    from concourse.bass2jax import (_bass_exec_p, install_neuronx_cc_hook,
                                    partition_id_tensor)
    nc = build(debug_outputs=(), upto="E", with_b1=with_b1, with_b2=with_b2,
               reps=reps)
    nc.compile()
    install_neuronx_cc_hook()
    partition_name = (nc.partition_id_tensor.name
                      if nc.partition_id_tensor else None)
    in_names, out_names, out_avals, zero_shapes = [], [], [], []
    for alloc in nc.m.functions[0].allocations:
        if not isinstance(alloc, mybir.MemoryLocationSet):
            continue
        name = alloc.memorylocations[0].name
        if alloc.kind == "ExternalInput":
            if name != partition_name:
                in_names.append(name)
        elif alloc.kind == "ExternalOutput":
            shape = tuple(alloc.tensor_shape)
            dtype = mybir.dt.np(alloc.dtype)
            out_names.append(name)
            out_avals.append(jax.core.ShapedArray(shape, dtype))
            zero_shapes.append((shape, dtype))
    n_params, n_outs = len(in_names), len(out_avals)
    all_in_names = in_names + out_names
    if partition_name is not None:
        all_in_names.append(partition_name)
    donate = tuple(range(n_params, n_params + n_outs))

    def _body(*args):
        operands = list(args)
        if partition_name is not None:
            operands.append(partition_id_tensor())
        outs = _bass_exec_p.bind(
            *operands, out_avals=tuple(out_avals), in_names=tuple(all_in_names),
            out_names=tuple(out_names), lowering_input_output_aliases=(),
            sim_require_finite=True, sim_require_nnan=True, nc=nc)
        return tuple(outs)

    devices = jax.devices()[:N_CORES]
    mesh = Mesh(np.asarray(devices), ("core",))
    sharded = jax.jit(
        shard_map(_body, mesh=mesh,
                  in_specs=(PartitionSpec("core"),) * (n_params + n_outs),
                  out_specs=(PartitionSpec("core"),) * n_outs,
                  check_rep=False),
        donate_argnums=donate, keep_unused=True)
    sharding = jax.sharding.NamedSharding(mesh, PartitionSpec("core"))
    state = {"in_names": in_names, "out_names": out_names,
             "zero_shapes": zero_shapes, "sharded": sharded,
             "sharding": sharding, "out_avals": out_avals}
    _CACHE[key] = state
    return state


def run_device(in_maps, with_b1, with_b2, reps=1):
    """Run the SPMD kernel; returns (per-core result dicts, wall seconds)."""
    import time as _time
    import jax
    st = _get_runner(with_b1, with_b2, reps)
    concat_in = [np.concatenate([np.asarray(m[name]) for m in in_maps], axis=0)
                 for name in st["in_names"]]
    dev_in = [jax.device_put(a, st["sharding"]) for a in concat_in]
    dev_zeros = [jax.device_put(
        np.zeros((N_CORES * s[0], *s[1:]), dt), st["sharding"])
        for (s, dt) in st["zero_shapes"]]
    for z in dev_zeros:
        z.block_until_ready()
    t0 = _time.perf_counter()
    out_arrs = st["sharded"](*dev_in, *dev_zeros)
    for o in out_arrs:
        o.block_until_ready()
    wall = _time.perf_counter() - t0
    results = [
        {name: np.asarray(out_arrs[i]).reshape(
            N_CORES, *st["out_avals"][i].shape)[c]
         for i, name in enumerate(st["out_names"])}
        for c in range(N_CORES)
    ]
    return results, wall


def kernel(**inputs):
    in_maps = host_prep(inputs)
    b1 = np.asarray(inputs["b_ih1"]) + np.asarray(inputs["b_hh1"])
    b2 = np.asarray(inputs["b_ih2"]) + np.asarray(inputs["b_hh2"])
    results, _ = run_device(in_maps, bool(np.any(b1)), bool(np.any(b2)))
    out = np.concatenate([results[c]["out"] for c in range(N_CORES)], axis=0)
    b_out = np.asarray(inputs["b_out"], np.float32)
    if np.any(b_out):
        out = out + b_out[None, None, :]
    return out

